# revision 19
# baseline (speedup 1.0000x reference)
"""Multi-head self-attention (batch=2, seq=2048, embed=1024, heads=16, causal)
sharded over 8 NeuronCores: data-parallel over batch (x2) and tensor-parallel
over heads (x4 groups of 4 heads).

Each core computes, for its (batch b, head group g):
  qkvT proj (transposed activations), causal softmax attention with the
  denominator folded into the AV matmul via a ones-column on V, and a partial
  output projection W_out[:, cols_g].T @ o_hat in transposed layout.
Host sums the 4 partials per batch, transposes back, and adds the constant
row  b_out + W_out @ b_v  (exact bias algebra).
"""

import os

import ml_dtypes
import numpy as np
from contextlib import ExitStack

import concourse.bass as bass
import concourse.mybir as mybir
import concourse.tile as tile
from concourse import bacc
from concourse.bass_utils import run_bass_kernel_spmd

N_HEADS = 16
EMBED = 1024
HEAD = 64
SEQ = 2048
BATCH = 2
N_CORES = 8
HPC = 4                # heads per core
GCOLS = HPC * HEAD     # 256 embed columns per head group
P = 128
CH = 512               # seq chunk
NCH = SEQ // CH        # 4
KT = SEQ // P          # 16 k tiles

DT = mybir.dt.float32
DTB = mybir.dt.bfloat16
DTH = mybir.dt.float16

LAST_EXEC_NS = None
LAST_RESULTS = None


def _build_program():
    nc = bacc.Bacc("TRN2", target_bir_lowering=False, debug=False,
                   num_devices=N_CORES)
    xT = nc.dram_tensor("xT", [EMBED, SEQ], DTB, kind="ExternalInput")
    wqkT = nc.dram_tensor("wqkT", [EMBED, 2 * GCOLS], DTB, kind="ExternalInput")
    wvT = nc.dram_tensor("wvT", [EMBED, GCOLS], DTB, kind="ExternalInput")
    bqk = nc.dram_tensor("bqk", [P, 4], DT, kind="ExternalInput")
    woT = nc.dram_tensor("woT", [GCOLS, EMBED], DTB, kind="ExternalInput")
    maskT = nc.dram_tensor("maskT", [P, 4 * CH], DTB, kind="ExternalInput")
    onesc = nc.dram_tensor("onesc", [P, KT * HPC], DTB, kind="ExternalInput")
    yT = nc.dram_tensor("yT", [EMBED, SEQ], DTH, kind="ExternalOutput")

    with tile.TileContext(nc) as tc, ExitStack() as ctx:
        const = ctx.enter_context(tc.tile_pool(name="const", bufs=1))
        xpool = ctx.enter_context(tc.tile_pool(name="xpool", bufs=3))
        stpool = ctx.enter_context(tc.tile_pool(name="stpool", bufs=6))
        small = ctx.enter_context(tc.tile_pool(name="small", bufs=8))
        outsb = ctx.enter_context(tc.tile_pool(name="outsb", bufs=3))
        ps512 = ctx.enter_context(tc.tile_pool(name="ps512", bufs=1, space="PSUM"))
        stP = ctx.enter_context(tc.tile_pool(name="stP", bufs=2, space="PSUM"))
        psO = ctx.enter_context(tc.tile_pool(name="psO", bufs=3, space="PSUM"))

        # ---- persistent SBUF residents (batched DMAs) ----
        wqkbig = const.tile([P, 8 * 2 * GCOLS], DTB, tag="wqkbig")
        for h_, eng in enumerate([nc.sync, nc.scalar, nc.gpsimd, nc.sync]):
            eng.dma_start(
                out=wqkbig.rearrange("p (i c) -> p i c", i=8)[:, 2 * h_:2 * h_ + 2],
                in_=wqkT.rearrange("(i p) c -> p i c", p=P)[:, 2 * h_:2 * h_ + 2])
        qt_t = [const.tile([P, SEQ], DTB, tag=f"qt{a}", name=f"qt{a}") for a in range(2)]
        kt_t = [const.tile([P, SEQ], DTB, tag=f"kt{a}", name=f"kt{a}") for a in range(2)]
        # v: [128, 16 ktiles x 4 heads x 65] with a ones column per head at 64
        VW = HPC * (HEAD + 1)
        vtbig = const.tile([P, KT * VW], DTB, tag="vtbig")
        nc.gpsimd.dma_start(
            out=vtbig.rearrange("p (t h d) -> p t h d",
                                t=KT, h=HPC)[:, :, :, HEAD:HEAD + 1],
            in_=onesc.rearrange("p (t h o) -> p t h o", t=KT, h=HPC))
        ohat_t = [const.tile([P, SEQ], DTB, tag=f"ohat{a}", name=f"ohat{a}") for a in range(2)]

        def load_x(qi, engines=None):
            engines = engines or [nc.sync, nc.scalar]
            n = len(engines)
            w = 8 // n
            sl = bass.ds(CH * qi, CH)
            xbig = xpool.tile([P, 8 * CH], DTB, tag="xt", name="xbig")
            for h_, eng in enumerate(engines):
                eng.dma_start(
                    out=xbig.rearrange("p (i c) -> p i c",
                                       i=8)[:, w * h_:w * h_ + w],
                    in_=xT.rearrange("(i p) s -> p i s",
                                     p=P)[:, w * h_:w * h_ + w, sl])
            return xbig

        def qkv_chunk(qi, xbig=None):
            sl = bass.ds(CH * qi, CH)
            if xbig is None:
                xbig = load_x(qi)
            for f in range(4):
                ps = ps512.tile([P, CH], DT, tag="ps512", name="ps512t")
                for i in range(8):
                    nc.tensor.matmul(
                        ps,
                        lhsT=wqkbig[:, bass.ds(2 * GCOLS * i + P * f, P)],
                        rhs=xbig[:, bass.ds(CH * i, CH)],
                        start=(i == 0), stop=(i == 7),
                    )
                dst = qt_t[f] if f < 2 else kt_t[f - 2]
                nc.vector.tensor_scalar_add(dst[:, sl], ps, bqk_sb[:, f:f + 1])
            for s in range(4):
                ti = 4 * qi + s
                ps = ps512.tile([P, GCOLS], DT, tag="ps512", name="ps512v")
                for i in range(8):
                    nc.tensor.matmul(
                        ps,
                        lhsT=xbig[:, bass.ds(CH * i + P * s, P)],
                        rhs=wvbig[:, bass.ds(GCOLS * i, GCOLS)],
                        start=(i == 0), stop=(i == 7),
                    )
                dst = vtbig[:, bass.ds(VW * ti, VW)].rearrange("p (h d) -> p h d", h=HPC)[:, :, 0:HEAD]
                src_ = ps.rearrange("p (h d) -> p h d", h=HPC)
                nc.vector.tensor_copy(dst, src_)

        def attn_pair(hp, qi):
            sl = bass.ds(CH * qi, CH)
            nk = 4 * qi + 4
            po = [psO.tile([HEAD + 1, CH], DT, tag="psO", name="psO")
                  for _ in range(2)]
            for ki in range(nk):
                pst = stP.tile([P, 2 * CH], DT, tag="stP", name="stP")
                for hh in range(2):
                    r0 = HEAD * hh
                    nc.tensor.matmul(
                        pst[:, bass.ds(CH * hh, CH)],
                        lhsT=kt_t[hp][r0:r0 + HEAD, bass.ds(P * ki, P)],
                        rhs=qt_t[hp][r0:r0 + HEAD, sl],
                        start=True, stop=True,
                    )
                st = stpool.tile([P, 2 * CH], DTB, tag="st", name="st")
                nc.scalar.activation(st[:], pst[:],
                                     mybir.ActivationFunctionType.Exp,
                                     scale=0.125)
                kr = ki - 4 * qi
                if kr >= 0:
                    for hh in range(2):
                        nc.vector.tensor_mul(
                            st[:, bass.ds(CH * hh, CH)],
                            st[:, bass.ds(CH * hh, CH)],
                            mask_sb[:, bass.ds(CH * kr, CH)])
                for hh in range(2):
                    h = 2 * hp + hh
                    nc.tensor.matmul(
                        po[hh],
                        lhsT=vtbig[:, bass.ds(VW * ki + (HEAD + 1) * h, HEAD + 1)],
                        rhs=st[:, bass.ds(CH * hh, CH)],
                        start=(ki == 0), stop=(ki == nk - 1),
                    )
            for hh in range(2):
                r0 = HEAD * hh
                den = small.tile([1, CH], DT, tag="den", name="den")
                nc.vector.tensor_copy(den[:], po[hh][HEAD:HEAD + 1, :])
                recip = small.tile([1, CH], DT, tag="recip", name="recip")
                nc.vector.reciprocal_approx_fast(recip[:], den[:])
                recipb = small.tile([HEAD, CH], DT, tag="recipb", name="recipb")
                nc.gpsimd.partition_broadcast(recipb[:], recip[:])
                nc.vector.tensor_mul(ohat_t[hp][r0:r0 + HEAD, sl],
                                     po[hh][0:HEAD, :], recipb[:])

        def outproj_chunk(qi, pool=None):
            pool = pool or ps512
            ptag = "stP" if pool is stP else "ps512"
            sl = bass.ds(CH * qi, CH)
            obig = outsb.tile([P, 8 * CH], DTH, tag="ot", name="obig")
            oengs = [nc.sync, nc.scalar, nc.gpsimd, nc.sync]
            for half in range(4):
                for m in range(2 * half, 2 * half + 2):
                    ps = pool.tile([P, CH], DT, tag=ptag, name="pso")
                    for k in range(2):
                        nc.tensor.matmul(
                            ps,
                            lhsT=wobig[:, bass.ds(EMBED * k + P * m, P)],
                            rhs=ohat_t[k][:, sl],
                            start=(k == 0), stop=(k == 1),
                        )
                    nc.vector.tensor_copy(obig[:, bass.ds(CH * m, CH)], ps)
                oengs[half].dma_start(
                    out=yT.rearrange("(m p) s -> p m s",
                                     p=P)[:, 2 * half:2 * half + 2, sl],
                    in_=obig.rearrange("p (m c) -> p m c",
                                       m=8)[:, 2 * half:2 * half + 2])

        # software pipeline: fill PE stalls at pair boundaries with
        # independent QKV / out-proj work
        x0 = load_x(0, engines=[nc.sync, nc.scalar, nc.gpsimd, nc.scalar])
        x1 = load_x(1, engines=[nc.sync, nc.scalar, nc.gpsimd, nc.scalar])
        wvbig = const.tile([P, 8 * GCOLS], DTB, tag="wvbig")
        for h_ in range(2):
            eng = nc.sync if h_ == 0 else nc.scalar
            eng.dma_start(
                out=wvbig.rearrange("p (i c) -> p i c", i=8)[:, 4 * h_:4 * h_ + 4],
                in_=wvT.rearrange("(i p) c -> p i c", p=P)[:, 4 * h_:4 * h_ + 4])
        wobig = const.tile([P, 2 * EMBED], DTB, tag="wobig")
        nc.scalar.dma_start(
            out=wobig.rearrange("p (k c) -> p k c", k=2),
            in_=woT.rearrange("(k p) c -> p k c", p=P))
        bqk_sb = const.tile([P, 4], DT, tag="bqk")
        nc.gpsimd.dma_start(out=bqk_sb, in_=bqk[:])
        mask_sb = const.tile([P, 4 * CH], DTB, tag="mask")
        nc.gpsimd.dma_start(out=mask_sb, in_=maskT[:])

        qkv_chunk(0, x0)
        attn_pair(0, 0)
        qkv_chunk(1, x1)
        attn_pair(1, 0)
        for qi in range(1, NCH):
            attn_pair(0, qi)
            outproj_chunk(qi - 1)
            if qi + 1 < NCH:
                qkv_chunk(qi + 1)
            attn_pair(1, qi)
        outproj_chunk(NCH - 1, pool=stP)

    nc.compile()
    return nc


def _make_masks():
    m = np.zeros((P, 4 * CH), dtype=np.float32)
    p = np.arange(P)[:, None]
    c = np.arange(CH)[None, :]
    for d in range(4):
        m[:, CH * d:CH * (d + 1)] = ((p + P * d) <= c).astype(np.float32)
    return m


def kernel(x, W_qkv, b_qkv, W_out, b_out):
    global LAST_EXEC_NS, LAST_RESULTS
    x = np.asarray(x, dtype=np.float32)
    W_qkv = np.asarray(W_qkv, dtype=np.float32)
    b_qkv = np.asarray(b_qkv, dtype=np.float32)
    W_out = np.asarray(W_out, dtype=np.float32)
    b_out = np.asarray(b_out, dtype=np.float32)

    nc = _build_program()
    masks = _make_masks()

    in_maps = []
    for c in range(N_CORES):
        b, g = divmod(c, HPC)
        q0 = GCOLS * g
        wq = W_qkv[q0:q0 + GCOLS]                    # [256, 1024]
        wk = W_qkv[EMBED + q0:EMBED + q0 + GCOLS]
        wv = W_qkv[2 * EMBED + q0:2 * EMBED + q0 + GCOLS]
        bq = b_qkv[q0:q0 + GCOLS]
        bk = b_qkv[EMBED + q0:EMBED + q0 + GCOLS]
        bqk = np.stack([bq[0:P], bq[P:2 * P], bk[0:P], bk[P:2 * P]],
                       axis=1).astype(np.float32)   # [128, 4]
        in_maps.append({
            "xT": np.ascontiguousarray(x[b].T).astype(ml_dtypes.bfloat16),
            "wqkT": np.ascontiguousarray(
                np.concatenate([wq, wk], 0).T).astype(ml_dtypes.bfloat16),
            "wvT": np.ascontiguousarray(wv.T).astype(ml_dtypes.bfloat16),
            "bqk": np.ascontiguousarray(bqk),
            "woT": np.ascontiguousarray(
                W_out[:, q0:q0 + GCOLS].T).astype(ml_dtypes.bfloat16),
            "maskT": masks.astype(ml_dtypes.bfloat16),
            "onesc": np.ones((P, KT * HPC), dtype=ml_dtypes.bfloat16),
        })

    want_trace = bool(int(os.environ.get("KTRACE", "0")))
    if want_trace:
        try:
            import antenv.axon_hooks  # noqa: F401
        except ImportError:
            want_trace = False
    res = run_bass_kernel_spmd(nc, in_maps, list(range(N_CORES)),
                               trace=want_trace,
                               tmpdir=os.environ.get("KTRACE_DIR") or None)
    LAST_EXEC_NS = res.exec_time_ns
    LAST_RESULTS = res

    out = np.empty((BATCH, SEQ, EMBED), dtype=np.float32)
    crow = (b_out + W_out @ b_qkv[2 * EMBED:]).astype(np.float32)
    for b in range(BATCH):
        acc = np.zeros((EMBED, SEQ), dtype=np.float32)
        for g in range(HPC):
            acc += res.results[HPC * b + g]["yT"].astype(np.float32)
        out[b] = acc.T + crow[None, :]
    return out


# revision 20
# speedup vs baseline: 1.0264x; 1.0264x over previous
"""Multi-head self-attention (batch=2, seq=2048, embed=1024, heads=16, causal)
sharded over 8 NeuronCores: data-parallel over batch (x2) and tensor-parallel
over heads (x4 groups of 4 heads).

Each core computes, for its (batch b, head group g):
  qkvT proj (transposed activations), causal softmax attention with the
  denominator folded into the AV matmul via a ones-column on V, and a partial
  output projection W_out[:, cols_g].T @ o_hat in transposed layout.
Host sums the 4 partials per batch, transposes back, and adds the constant
row  b_out + W_out @ b_v  (exact bias algebra).
"""

import os

import ml_dtypes
import numpy as np
from contextlib import ExitStack

import concourse.bass as bass
import concourse.mybir as mybir
import concourse.tile as tile
from concourse import bacc
from concourse.bass_utils import run_bass_kernel_spmd

N_HEADS = 16
EMBED = 1024
HEAD = 64
SEQ = 2048
BATCH = 2
N_CORES = 8
HPC = 4                # heads per core
GCOLS = HPC * HEAD     # 256 embed columns per head group
P = 128
CH = 512               # seq chunk
NCH = SEQ // CH        # 4
KT = SEQ // P          # 16 k tiles

DT = mybir.dt.float32
DTB = mybir.dt.bfloat16
DTH = mybir.dt.float16

LAST_EXEC_NS = None
LAST_RESULTS = None


def _build_program():
    nc = bacc.Bacc("TRN2", target_bir_lowering=False, debug=False,
                   num_devices=N_CORES)
    xT = nc.dram_tensor("xT", [EMBED, SEQ], DTB, kind="ExternalInput")
    wqkT = nc.dram_tensor("wqkT", [EMBED, 2 * GCOLS], DTB, kind="ExternalInput")
    wvT = nc.dram_tensor("wvT", [EMBED, GCOLS], DTB, kind="ExternalInput")
    bqk = nc.dram_tensor("bqk", [P, 4], DT, kind="ExternalInput")
    woT = nc.dram_tensor("woT", [GCOLS, EMBED], DTB, kind="ExternalInput")
    maskT = nc.dram_tensor("maskT", [P, 4 * CH], DTB, kind="ExternalInput")
    onesc = nc.dram_tensor("onesc", [P, KT * HPC], DTB, kind="ExternalInput")
    yT = nc.dram_tensor("yT", [EMBED, SEQ], DTH, kind="ExternalOutput")

    with tile.TileContext(nc) as tc, ExitStack() as ctx:
        const = ctx.enter_context(tc.tile_pool(name="const", bufs=1))
        xpool = ctx.enter_context(tc.tile_pool(name="xpool", bufs=3))
        stpool = ctx.enter_context(tc.tile_pool(name="stpool", bufs=6))
        small = ctx.enter_context(tc.tile_pool(name="small", bufs=8))
        outsb = ctx.enter_context(tc.tile_pool(name="outsb", bufs=3))
        ps512 = ctx.enter_context(tc.tile_pool(name="ps512", bufs=1, space="PSUM"))
        stP = ctx.enter_context(tc.tile_pool(name="stP", bufs=2, space="PSUM"))
        psO = ctx.enter_context(tc.tile_pool(name="psO", bufs=3, space="PSUM"))

        # ---- persistent SBUF residents (batched DMAs) ----
        wqkbig = const.tile([P, 8 * 2 * GCOLS], DTB, tag="wqkbig")

        def load_wqk_piece(h_, eng):
            eng.dma_start(
                out=wqkbig.rearrange("p (i c) -> p i c", i=8)[:, 2 * h_:2 * h_ + 2],
                in_=wqkT.rearrange("(i p) c -> p i c", p=P)[:, 2 * h_:2 * h_ + 2])

        def load_x_piece(xbig, qi, h_, eng):
            sl = bass.ds(CH * qi, CH)
            eng.dma_start(
                out=xbig.rearrange("p (i c) -> p i c", i=8)[:, 2 * h_:2 * h_ + 2],
                in_=xT.rearrange("(i p) s -> p i s", p=P)[:, 2 * h_:2 * h_ + 2, sl])
        qt_t = [const.tile([P, SEQ], DTB, tag=f"qt{a}", name=f"qt{a}") for a in range(2)]
        kt_t = [const.tile([P, SEQ], DTB, tag=f"kt{a}", name=f"kt{a}") for a in range(2)]
        # v: [128, 16 ktiles x 4 heads x 65] with a ones column per head at 64
        VW = HPC * (HEAD + 1)
        vtbig = const.tile([P, KT * VW], DTB, tag="vtbig")
        nc.gpsimd.dma_start(
            out=vtbig.rearrange("p (t h d) -> p t h d",
                                t=KT, h=HPC)[:, :, :, HEAD:HEAD + 1],
            in_=onesc.rearrange("p (t h o) -> p t h o", t=KT, h=HPC))
        ohat_t = [const.tile([P, SEQ], DTB, tag=f"ohat{a}", name=f"ohat{a}") for a in range(2)]

        def load_x(qi, engines=None):
            engines = engines or [nc.sync, nc.scalar]
            n = len(engines)
            w = 8 // n
            sl = bass.ds(CH * qi, CH)
            xbig = xpool.tile([P, 8 * CH], DTB, tag="xt", name="xbig")
            for h_, eng in enumerate(engines):
                eng.dma_start(
                    out=xbig.rearrange("p (i c) -> p i c",
                                       i=8)[:, w * h_:w * h_ + w],
                    in_=xT.rearrange("(i p) s -> p i s",
                                     p=P)[:, w * h_:w * h_ + w, sl])
            return xbig

        def qkv_chunk(qi, xbig=None):
            sl = bass.ds(CH * qi, CH)
            if xbig is None:
                xbig = load_x(qi)
            for f in range(4):
                ps = ps512.tile([P, CH], DT, tag="ps512", name="ps512t")
                for i in range(8):
                    nc.tensor.matmul(
                        ps,
                        lhsT=wqkbig[:, bass.ds(2 * GCOLS * i + P * f, P)],
                        rhs=xbig[:, bass.ds(CH * i, CH)],
                        start=(i == 0), stop=(i == 7),
                    )
                dst = qt_t[f] if f < 2 else kt_t[f - 2]
                nc.vector.tensor_scalar_add(dst[:, sl], ps, bqk_sb[:, f:f + 1])
            for s in range(4):
                ti = 4 * qi + s
                ps = ps512.tile([P, GCOLS], DT, tag="ps512", name="ps512v")
                for i in range(8):
                    nc.tensor.matmul(
                        ps,
                        lhsT=xbig[:, bass.ds(CH * i + P * s, P)],
                        rhs=wvbig[:, bass.ds(GCOLS * i, GCOLS)],
                        start=(i == 0), stop=(i == 7),
                    )
                dst = vtbig[:, bass.ds(VW * ti, VW)].rearrange("p (h d) -> p h d", h=HPC)[:, :, 0:HEAD]
                src_ = ps.rearrange("p (h d) -> p h d", h=HPC)
                nc.vector.tensor_copy(dst, src_)

        def attn_pair(hp, qi):
            sl = bass.ds(CH * qi, CH)
            nk = 4 * qi + 4
            po = [psO.tile([HEAD + 1, CH], DT, tag="psO", name="psO")
                  for _ in range(2)]
            for ki in range(nk):
                pst = stP.tile([P, 2 * CH], DT, tag="stP", name="stP")
                for hh in range(2):
                    r0 = HEAD * hh
                    nc.tensor.matmul(
                        pst[:, bass.ds(CH * hh, CH)],
                        lhsT=kt_t[hp][r0:r0 + HEAD, bass.ds(P * ki, P)],
                        rhs=qt_t[hp][r0:r0 + HEAD, sl],
                        start=True, stop=True,
                    )
                st = stpool.tile([P, 2 * CH], DTB, tag="st", name="st")
                nc.scalar.activation(st[:], pst[:],
                                     mybir.ActivationFunctionType.Exp,
                                     scale=0.125)
                kr = ki - 4 * qi
                if kr >= 0:
                    for hh in range(2):
                        nc.vector.tensor_mul(
                            st[:, bass.ds(CH * hh, CH)],
                            st[:, bass.ds(CH * hh, CH)],
                            mask_sb[:, bass.ds(CH * kr, CH)])
                for hh in range(2):
                    h = 2 * hp + hh
                    nc.tensor.matmul(
                        po[hh],
                        lhsT=vtbig[:, bass.ds(VW * ki + (HEAD + 1) * h, HEAD + 1)],
                        rhs=st[:, bass.ds(CH * hh, CH)],
                        start=(ki == 0), stop=(ki == nk - 1),
                    )
            for hh in range(2):
                r0 = HEAD * hh
                den = small.tile([1, CH], DT, tag="den", name="den")
                nc.vector.tensor_copy(den[:], po[hh][HEAD:HEAD + 1, :])
                recip = small.tile([1, CH], DT, tag="recip", name="recip")
                nc.vector.reciprocal_approx_fast(recip[:], den[:])
                recipb = small.tile([HEAD, CH], DT, tag="recipb", name="recipb")
                nc.gpsimd.partition_broadcast(recipb[:], recip[:])
                nc.vector.tensor_mul(ohat_t[hp][r0:r0 + HEAD, sl],
                                     po[hh][0:HEAD, :], recipb[:])

        def outproj_chunk(qi, pool=None):
            pool = pool or ps512
            ptag = "stP" if pool is stP else "ps512"
            sl = bass.ds(CH * qi, CH)
            obig = outsb.tile([P, 8 * CH], DTH, tag="ot", name="obig")
            oengs = [nc.sync, nc.scalar, nc.gpsimd, nc.sync]
            for half in range(4):
                for m in range(2 * half, 2 * half + 2):
                    ps = pool.tile([P, CH], DT, tag=ptag, name="pso")
                    for k in range(2):
                        nc.tensor.matmul(
                            ps,
                            lhsT=wobig[:, bass.ds(EMBED * k + P * m, P)],
                            rhs=ohat_t[k][:, sl],
                            start=(k == 0), stop=(k == 1),
                        )
                    nc.vector.tensor_copy(obig[:, bass.ds(CH * m, CH)], ps)
                oengs[half].dma_start(
                    out=yT.rearrange("(m p) s -> p m s",
                                     p=P)[:, 2 * half:2 * half + 2, sl],
                    in_=obig.rearrange("p (m c) -> p m c",
                                       m=8)[:, 2 * half:2 * half + 2])

        # software pipeline: fill PE stalls at pair boundaries with
        # independent QKV / out-proj work
        x0 = xpool.tile([P, 8 * CH], DTB, tag="xt", name="x0big")
        for h_ in range(4):
            load_wqk_piece(h_, nc.sync if h_ % 2 == 0 else nc.scalar)
            load_x_piece(x0, 0, h_, nc.scalar if h_ % 2 == 0 else nc.sync)
        wvbig = const.tile([P, 8 * GCOLS], DTB, tag="wvbig")
        for h_ in range(2):
            eng = nc.sync if h_ == 0 else nc.scalar
            eng.dma_start(
                out=wvbig.rearrange("p (i c) -> p i c", i=8)[:, 4 * h_:4 * h_ + 4],
                in_=wvT.rearrange("(i p) c -> p i c", p=P)[:, 4 * h_:4 * h_ + 4])
        x1 = load_x(1, engines=[nc.sync, nc.scalar, nc.sync, nc.scalar])
        wobig = const.tile([P, 2 * EMBED], DTB, tag="wobig")
        nc.scalar.dma_start(
            out=wobig.rearrange("p (k c) -> p k c", k=2),
            in_=woT.rearrange("(k p) c -> p k c", p=P))
        bqk_sb = const.tile([P, 4], DT, tag="bqk")
        nc.gpsimd.dma_start(out=bqk_sb, in_=bqk[:])
        mask_sb = const.tile([P, 4 * CH], DTB, tag="mask")
        nc.gpsimd.dma_start(out=mask_sb, in_=maskT[:])

        qkv_chunk(0, x0)
        attn_pair(0, 0)
        qkv_chunk(1, x1)
        attn_pair(1, 0)
        for qi in range(1, NCH):
            attn_pair(0, qi)
            outproj_chunk(qi - 1)
            if qi + 1 < NCH:
                qkv_chunk(qi + 1)
            attn_pair(1, qi)
        outproj_chunk(NCH - 1, pool=stP)

    nc.compile()
    return nc


def _make_masks():
    m = np.zeros((P, 4 * CH), dtype=np.float32)
    p = np.arange(P)[:, None]
    c = np.arange(CH)[None, :]
    for d in range(4):
        m[:, CH * d:CH * (d + 1)] = ((p + P * d) <= c).astype(np.float32)
    return m


def kernel(x, W_qkv, b_qkv, W_out, b_out):
    global LAST_EXEC_NS, LAST_RESULTS
    x = np.asarray(x, dtype=np.float32)
    W_qkv = np.asarray(W_qkv, dtype=np.float32)
    b_qkv = np.asarray(b_qkv, dtype=np.float32)
    W_out = np.asarray(W_out, dtype=np.float32)
    b_out = np.asarray(b_out, dtype=np.float32)

    nc = _build_program()
    masks = _make_masks()

    in_maps = []
    for c in range(N_CORES):
        b, g = divmod(c, HPC)
        q0 = GCOLS * g
        wq = W_qkv[q0:q0 + GCOLS]                    # [256, 1024]
        wk = W_qkv[EMBED + q0:EMBED + q0 + GCOLS]
        wv = W_qkv[2 * EMBED + q0:2 * EMBED + q0 + GCOLS]
        bq = b_qkv[q0:q0 + GCOLS]
        bk = b_qkv[EMBED + q0:EMBED + q0 + GCOLS]
        bqk = np.stack([bq[0:P], bq[P:2 * P], bk[0:P], bk[P:2 * P]],
                       axis=1).astype(np.float32)   # [128, 4]
        in_maps.append({
            "xT": np.ascontiguousarray(x[b].T).astype(ml_dtypes.bfloat16),
            "wqkT": np.ascontiguousarray(
                np.concatenate([wq, wk], 0).T).astype(ml_dtypes.bfloat16),
            "wvT": np.ascontiguousarray(wv.T).astype(ml_dtypes.bfloat16),
            "bqk": np.ascontiguousarray(bqk),
            "woT": np.ascontiguousarray(
                W_out[:, q0:q0 + GCOLS].T).astype(ml_dtypes.bfloat16),
            "maskT": masks.astype(ml_dtypes.bfloat16),
            "onesc": np.ones((P, KT * HPC), dtype=ml_dtypes.bfloat16),
        })

    want_trace = bool(int(os.environ.get("KTRACE", "0")))
    if want_trace:
        try:
            import antenv.axon_hooks  # noqa: F401
        except ImportError:
            want_trace = False
    res = run_bass_kernel_spmd(nc, in_maps, list(range(N_CORES)),
                               trace=want_trace,
                               tmpdir=os.environ.get("KTRACE_DIR") or None)
    LAST_EXEC_NS = res.exec_time_ns
    LAST_RESULTS = res

    out = np.empty((BATCH, SEQ, EMBED), dtype=np.float32)
    crow = (b_out + W_out @ b_qkv[2 * EMBED:]).astype(np.float32)
    for b in range(BATCH):
        acc = np.zeros((EMBED, SEQ), dtype=np.float32)
        for g in range(HPC):
            acc += res.results[HPC * b + g]["yT"].astype(np.float32)
        out[b] = acc.T + crow[None, :]
    return out
